# revision 13
# baseline (speedup 1.0000x reference)
"""Atomwise (segment_reduce) Trainium2 kernel — v2.

y[m] = sum_{atoms i in molecule m} (x[i] . W[0] + b[0]),  m in [0, 100000)

8 NeuronCores, SPMD, no collectives.  The bias is folded into x on the
host (x += W0*b0/|W0|^2), so the device computes a pure projected
segment-sum.  x is quantized to fp8 e3m4 (halves HBM traffic vs bf16;
rel-err ~1.0e-2 on this data).

Geometry is fully static and identical on all cores (one shared SPMD
graph): each core owns a fixed 250_000-atom range, split into NCH=123
windows of 2048 atoms (16 blocks of 128).  Windows cut mid-molecule;
a molecule split across blocks/windows/cores is summed on the host
during unpack (np.add.at over ~15k entries).

Device pipeline per 2048-atom chunk:
  * grouped DMA of fp8 X windows (contiguous per partition)
  * one DVE is_equal: H[atom, (block, slot)] one-hot of block-relative
    molecule slot (lidx broadcast vs iota), BF=16 slots per block
  * 16 matmuls: stationary X_j [128 atoms x 128 feats] fp8 (FWL),
    moving H_j [128 x BF] -> PSUM S^T[128 feats, 256 slots] (disjoint
    per-block slices)
  * ScalarE copies S^T -> SBUF bf16
  * 2 small matmuls project: y_slots = S^T^T @ w0  -> PSUM [128, 2]
  * DVE copies into y_all [128, NCH*2]
One output DMA at the end; host scatters slots into molecules.
"""

import numpy as np
import ml_dtypes

N_ATOMS = 2_000_000
N_IN = 128
N_MOL = 100_000
NCORES = 8
P = 128
PC = N_ATOMS // NCORES      # atoms per core
NB = 16                     # 128-atom blocks per window
A_max = NB * P              # 2048 atoms per window
NCH = (PC + A_max - 1) // A_max   # 123 windows
NPAD = NCH * A_max
BF = 16                     # molecule slots per block (max span 10 on data)
SLOTS = NB * BF             # 256 slots per chunk
NYC = SLOTS // P            # 2 projection matmuls / output cols per chunk

_graph_cache: dict = {}


def _chunk_groups(nch):
    groups, c = [], 0
    for sz in (16, 8, 4, 2, 1):
        while nch - c >= sz:
            groups.append((c, sz))
            c += sz
    return groups


def _build_graph():
    import concourse.mybir as mybir
    from concourse import bacc
    from concourse.tile import TileContext

    f32 = mybir.dt.float32
    bf16 = mybir.dt.bfloat16
    f8 = mybir.dt.float8e3

    IOTA_OFF = 0
    LIDX_OFF = SLOTS
    W0_OFF = LIDX_OFF + NCH * NB
    W0_OFF += W0_OFF % 2
    CW = W0_OFF + 4

    nc = bacc.Bacc()
    xw = nc.dram_tensor("xw", [NPAD, N_IN], f8, kind="ExternalInput")
    cst = nc.dram_tensor("cst", [P, CW], f8, kind="ExternalInput")
    out = nc.dram_tensor("out", [P * NCH * NYC], f32, kind="ExternalOutput")
    out_r = out.rearrange("(p c) -> p c", c=NCH * NYC)
    groups = _chunk_groups(NCH)

    with TileContext(nc) as tc:
        with tc.tile_pool(name="const", bufs=1) as cpool, \
             tc.tile_pool(name="xbp", bufs=4) as xbpool, \
             tc.tile_pool(name="hp", bufs=6) as hpool, \
             tc.tile_pool(name="scp", bufs=6) as scpool, \
             tc.tile_pool(name="pp", bufs=5, space="PSUM") as pspool, \
             tc.tile_pool(name="yp", bufs=1, space="PSUM") as ypool:
            cst_t = cpool.tile([P, CW], f8)
            yp_all = ypool.tile([P, 512], f32)
            w0_col = cst_t[:, W0_OFF:W0_OFF + 2].bitcast(bf16)

            PROJ_DELAY = 2
            EQ_LEAD = 3
            ht_tiles = {}

            def _emit_eq(c):
                ht = hpool.tile([P, SLOTS], bf16, tag="h")
                ht_tiles[c] = ht
                nc.vector.tensor_tensor(
                    out=ht[:],
                    in0=cst_t[:, LIDX_OFF + c * NB:
                              LIDX_OFF + (c + 1) * NB
                              ].to_broadcast([P, NB, BF]),
                    in1=cst_t[:, IOTA_OFF:IOTA_OFF + SLOTS],
                    op=mybir.AluOpType.is_equal)

            def _emit_proj(c, sc):
                for k in range(NYC):
                    nc.tensor.matmul(
                        yp_all[:, c * NYC + k:c * NYC + k + 1],
                        lhsT=sc[:, k * P:(k + 1) * P],
                        rhs=w0_col[:, 0:1],
                        start=True,
                        stop=True,
                    )

            chunk_xq = {}
            for gstart, gc in groups:
                xq = None  # placeholder; created at emission time
                for cc in range(gc):
                    chunk_xq[gstart + cc] = (gstart, gc, cc)

            group_tiles = {}
            proj_q = []
            gidx = 0
            for c in range(NCH):
                gstart, gc, cc = chunk_xq[c]
                if cc == 0:
                    if gstart == 0:
                        nc.sync.dma_start(cst_t[:], cst[:, :])
                    xq = xbpool.tile([P, gc * NB * N_IN], f8, tag="xq")
                    group_tiles[gstart] = xq
                    nc.sync.dma_start(
                        xq[:],
                        xw[gstart * A_max:(gstart + gc) * A_max, :].rearrange(
                            "(p j) f -> p (j f)", p=P),
                    )
                    gidx += 1
                    if gstart == 0:
                        for ce in range(min(EQ_LEAD + 1, NCH)):
                            _emit_eq(ce)
                xq = group_tiles[gstart]
                ht = ht_tiles.pop(c)
                ps = pspool.tile([P, 512], f32, tag="ps")
                for j in range(NB):
                    nc.tensor.matmul(
                        ps[:, j * BF:(j + 1) * BF],
                        lhsT=xq[:, (cc * NB + j) * N_IN:
                                (cc * NB + j + 1) * N_IN],
                        rhs=ht[:, j * BF:(j + 1) * BF],
                        start=True,
                        stop=True,
                    )
                sc = scpool.tile([P, SLOTS], bf16, tag="sc")
                nc.scalar.activation(
                    sc[:], ps[:, 0:SLOTS],
                    mybir.ActivationFunctionType.Copy)
                if c + EQ_LEAD + 1 < NCH:
                    _emit_eq(c + EQ_LEAD + 1)
                proj_q.append((c, sc))
                if len(proj_q) > PROJ_DELAY:
                    _emit_proj(*proj_q.pop(0))
            while proj_q:
                _emit_proj(*proj_q.pop(0))
            y_sb = cpool.tile([P, NCH * NYC], f32)
            nc.vector.tensor_copy(y_sb[:], yp_all[:, 0:NCH * NYC])
            nc.sync.dma_start(out_r[:, :], y_sb[:])
    nc.finalize()
    return nc


def _prep(inputs):
    x = np.asarray(inputs["scalar_representation"], dtype=np.float32)
    idx = np.asarray(inputs["idx_m"]).astype(np.int64)
    W = np.asarray(inputs["W"], dtype=np.float32)
    b = np.asarray(inputs["b"], dtype=np.float32)

    # fold the bias into x: (x + v) . w0 == x . w0 + b0
    v = W[0] * (b[0] / np.dot(W[0], W[0]))

    IOTA_OFF = 0
    LIDX_OFF = SLOTS
    W0_OFF = LIDX_OFF + NCH * NB
    W0_OFF += W0_OFF % 2
    CW = W0_OFF + 4
    iota_row = np.tile(np.arange(BF, dtype=np.float32), NB).astype(
        ml_dtypes.float8_e3m4)
    groups = _chunk_groups(NCH)

    in_maps = []
    unpack = []
    for i in range(NCORES):
        idxc = idx[i * PC:(i + 1) * PC]
        q8 = np.zeros((NPAD, N_IN), dtype=ml_dtypes.float8_e3m4)
        q8[:PC] = (x[i * PC:(i + 1) * PC] + v).astype(ml_dtypes.float8_e3m4)

        kb = np.arange(NCH * NB, dtype=np.int64) * P
        valid = kb < PC
        base = np.zeros(NCH * NB, dtype=np.int64)
        base[valid] = idxc[kb[valid]]
        ke = np.minimum(kb + P - 1, PC - 1)
        span = np.zeros(NCH * NB, dtype=np.int64)
        span[valid] = idxc[ke[valid]] - base[valid] + 1
        assert span.max() <= BF, f"block span {span.max()} > BF={BF}"

        lidx = np.full(NPAD, -1.0, dtype=np.float32)
        lidx[:PC] = (idxc - np.repeat(base, P)[:PC]).astype(np.float32)
        lidx_t = lidx.reshape(NCH, NB, P).transpose(2, 0, 1).reshape(
            P, NCH * NB).astype(ml_dtypes.float8_e3m4)

        parts = []
        for gstart, gc in groups:
            blk = q8[gstart * A_max:(gstart + gc) * A_max]
            parts.append(np.ascontiguousarray(
                blk.reshape(gc, NB, P, N_IN).transpose(2, 0, 1, 3)
                   .reshape(gc * A_max, N_IN)))
        xw_i = np.concatenate(parts, axis=0)

        cst = np.zeros((P, CW), dtype=ml_dtypes.float8_e3m4)
        cst[:, IOTA_OFF:IOTA_OFF + SLOTS] = iota_row[None, :]
        cst[:, LIDX_OFF:LIDX_OFF + NCH * NB] = lidx_t
        w0b = W[0].astype(ml_dtypes.bfloat16)[:, None].view(
            ml_dtypes.float8_e3m4)
        cst[:, W0_OFF:W0_OFF + 2] = w0b
        in_maps.append({"xw": xw_i, "cst": np.ascontiguousarray(cst)})

        # unpack tables: (partition, column, molecule) per live slot
        nblk = int(valid.sum())
        sp = span[:nblk]
        tot = int(sp.sum())
        starts = np.zeros(nblk, dtype=np.int64)
        starts[1:] = np.cumsum(sp)[:-1]
        blk_of = np.repeat(np.arange(nblk, dtype=np.int64), sp)
        s_off = np.arange(tot, dtype=np.int64) - np.repeat(starts, sp)
        mol = np.repeat(base[:nblk], sp) + s_off
        slotfull = (blk_of % NB) * BF + s_off
        chunkc = blk_of // NB
        p_idx = slotfull % P
        col_idx = chunkc * NYC + slotfull // P
        unpack.append((p_idx, col_idx, mol))
    return in_maps, unpack


def _run(inputs, trace=False):
    from concourse import bass_utils

    in_maps, unpack = _prep(inputs)
    key = (NCH, BF)
    if key not in _graph_cache:
        _graph_cache[key] = _build_graph()
    nc = _graph_cache[key]

    res = bass_utils.run_bass_kernel_spmd(
        nc, in_maps, core_ids=list(range(NCORES)), trace=trace
    )
    y = np.zeros(N_MOL, dtype=np.float32)
    for i in range(NCORES):
        arr = res.results[i]["out"].reshape(P, NCH * NYC)
        p_idx, col_idx, mol = unpack[i]
        np.add.at(y, mol, arr[p_idx, col_idx])
    return y, res


def kernel(**inputs) -> np.ndarray:
    y, _ = _run(inputs, trace=False)
    return y


# revision 15
# speedup vs baseline: 1.0325x; 1.0325x over previous
"""Atomwise (segment_reduce) Trainium2 kernel — v2.

y[m] = sum_{atoms i in molecule m} (x[i] . W[0] + b[0]),  m in [0, 100000)

8 NeuronCores, SPMD, no collectives.  The bias is folded into x on the
host (x += W0*b0/|W0|^2), so the device computes a pure projected
segment-sum.  x is quantized to fp8 e3m4 (halves HBM traffic vs bf16;
rel-err ~1.0e-2 on this data).

Geometry is fully static and identical on all cores (one shared SPMD
graph): each core owns a fixed 250_000-atom range, split into NCH=123
windows of 2048 atoms (16 blocks of 128).  Windows cut mid-molecule;
a molecule split across blocks/windows/cores is summed on the host
during unpack (np.add.at over ~15k entries).

Device pipeline per 2048-atom chunk:
  * grouped DMA of fp8 X windows (contiguous per partition)
  * one DVE is_equal: H[atom, (block, slot)] one-hot of block-relative
    molecule slot (lidx broadcast vs iota), BF=16 slots per block
  * 16 matmuls: stationary X_j [128 atoms x 128 feats] fp8 (FWL),
    moving H_j [128 x BF] -> PSUM S^T[128 feats, 256 slots] (disjoint
    per-block slices)
  * ScalarE copies S^T -> SBUF bf16
  * 2 small matmuls project: y_slots = S^T^T @ w0  -> PSUM [128, 2]
  * DVE copies into y_all [128, NCH*2]
One output DMA at the end; host scatters slots into molecules.
"""

import numpy as np
import ml_dtypes

N_ATOMS = 2_000_000
N_IN = 128
N_MOL = 100_000
NCORES = 8
P = 128
PC = N_ATOMS // NCORES      # atoms per core
NB = 16                     # 128-atom blocks per window
A_max = NB * P              # 2048 atoms per window
NCH = (PC + A_max - 1) // A_max   # 123 windows
NPAD = NCH * A_max
BF = 16                     # molecule slots per block (max span 10 on data)
SLOTS = NB * BF             # 256 slots per chunk
NYC = SLOTS // P            # 2 projection matmuls / output cols per chunk

_graph_cache: dict = {}


def _chunk_groups(nch):
    groups, c = [], 0
    for sz in (2, 2, 4):
        if nch - c >= sz:
            groups.append((c, sz))
            c += sz
    for sz in (8, 4, 2, 1):
        while nch - c >= sz:
            groups.append((c, sz))
            c += sz
    return groups


def _build_graph():
    import concourse.mybir as mybir
    from concourse import bacc
    from concourse.tile import TileContext

    f32 = mybir.dt.float32
    bf16 = mybir.dt.bfloat16
    f8 = mybir.dt.float8e3

    IOTA_OFF = 0
    LIDX_OFF = SLOTS
    W0_OFF = LIDX_OFF + NCH * NB
    W0_OFF += W0_OFF % 2
    CW = W0_OFF + 4

    nc = bacc.Bacc()
    xw = nc.dram_tensor("xw", [NPAD, N_IN], f8, kind="ExternalInput")
    cst = nc.dram_tensor("cst", [P, CW], f8, kind="ExternalInput")
    out = nc.dram_tensor("out", [P * NCH * NYC], f32, kind="ExternalOutput")
    out_r = out.rearrange("(p c) -> p c", c=NCH * NYC)
    groups = _chunk_groups(NCH)

    with TileContext(nc) as tc:
        with tc.tile_pool(name="const", bufs=1) as cpool, \
             tc.tile_pool(name="xbp", bufs=10) as xbpool, \
             tc.tile_pool(name="hp", bufs=6) as hpool, \
             tc.tile_pool(name="scp", bufs=6) as scpool, \
             tc.tile_pool(name="pp", bufs=5, space="PSUM") as pspool, \
             tc.tile_pool(name="yp", bufs=1, space="PSUM") as ypool:
            cst_t = cpool.tile([P, CW], f8)
            yp_all = ypool.tile([P, 512], f32)
            w0_col = cst_t[:, W0_OFF:W0_OFF + 2].bitcast(bf16)

            PROJ_DELAY = 2
            EQ_LEAD = 3
            ht_tiles = {}

            def _emit_eq(c):
                ht = hpool.tile([P, SLOTS], bf16, tag="h")
                ht_tiles[c] = ht
                nc.vector.tensor_tensor(
                    out=ht[:],
                    in0=cst_t[:, LIDX_OFF + c * NB:
                              LIDX_OFF + (c + 1) * NB
                              ].to_broadcast([P, NB, BF]),
                    in1=cst_t[:, IOTA_OFF:IOTA_OFF + SLOTS],
                    op=mybir.AluOpType.is_equal)

            def _emit_proj(c, sc):
                for k in range(NYC):
                    nc.tensor.matmul(
                        yp_all[:, c * NYC + k:c * NYC + k + 1],
                        lhsT=sc[:, k * P:(k + 1) * P],
                        rhs=w0_col[:, 0:1],
                        start=True,
                        stop=True,
                    )

            chunk_xq = {}
            for gstart, gc in groups:
                xq = None  # placeholder; created at emission time
                for cc in range(gc):
                    chunk_xq[gstart + cc] = (gstart, gc, cc)

            group_tiles = {}
            proj_q = []
            gidx = 0
            for c in range(NCH):
                gstart, gc, cc = chunk_xq[c]
                if cc == 0:
                    if gstart == 0:
                        nc.sync.dma_start(cst_t[:], cst[:, :])
                    xq = xbpool.tile([P, gc * NB * N_IN], f8, tag="xq")
                    group_tiles[gstart] = xq
                    nc.sync.dma_start(
                        xq[:],
                        xw[gstart * A_max:(gstart + gc) * A_max, :].rearrange(
                            "(p j) f -> p (j f)", p=P),
                    )
                    gidx += 1
                    if gstart == 0:
                        for ce in range(min(EQ_LEAD + 1, NCH)):
                            _emit_eq(ce)
                xq = group_tiles[gstart]
                ht = ht_tiles.pop(c)
                ps = pspool.tile([P, 512], f32, tag="ps")
                for j in range(NB):
                    nc.tensor.matmul(
                        ps[:, j * BF:(j + 1) * BF],
                        lhsT=xq[:, (cc * NB + j) * N_IN:
                                (cc * NB + j + 1) * N_IN],
                        rhs=ht[:, j * BF:(j + 1) * BF],
                        start=True,
                        stop=True,
                    )
                sc = scpool.tile([P, SLOTS], bf16, tag="sc")
                nc.scalar.activation(
                    sc[:], ps[:, 0:SLOTS],
                    mybir.ActivationFunctionType.Copy)
                if c + EQ_LEAD + 1 < NCH:
                    _emit_eq(c + EQ_LEAD + 1)
                proj_q.append((c, sc))
                if len(proj_q) > PROJ_DELAY:
                    _emit_proj(*proj_q.pop(0))
            while proj_q:
                _emit_proj(*proj_q.pop(0))
            y_sb = cpool.tile([P, NCH * NYC], f32)
            nc.vector.tensor_copy(y_sb[:], yp_all[:, 0:NCH * NYC])
            nc.sync.dma_start(out_r[:, :], y_sb[:])
    nc.finalize()
    return nc


def _prep(inputs):
    x = np.asarray(inputs["scalar_representation"], dtype=np.float32)
    idx = np.asarray(inputs["idx_m"]).astype(np.int64)
    W = np.asarray(inputs["W"], dtype=np.float32)
    b = np.asarray(inputs["b"], dtype=np.float32)

    # fold the bias into x: (x + v) . w0 == x . w0 + b0
    v = W[0] * (b[0] / np.dot(W[0], W[0]))

    IOTA_OFF = 0
    LIDX_OFF = SLOTS
    W0_OFF = LIDX_OFF + NCH * NB
    W0_OFF += W0_OFF % 2
    CW = W0_OFF + 4
    iota_row = np.tile(np.arange(BF, dtype=np.float32), NB).astype(
        ml_dtypes.float8_e3m4)
    groups = _chunk_groups(NCH)

    in_maps = []
    unpack = []
    for i in range(NCORES):
        idxc = idx[i * PC:(i + 1) * PC]
        q8 = np.zeros((NPAD, N_IN), dtype=ml_dtypes.float8_e3m4)
        q8[:PC] = (x[i * PC:(i + 1) * PC] + v).astype(ml_dtypes.float8_e3m4)

        kb = np.arange(NCH * NB, dtype=np.int64) * P
        valid = kb < PC
        base = np.zeros(NCH * NB, dtype=np.int64)
        base[valid] = idxc[kb[valid]]
        ke = np.minimum(kb + P - 1, PC - 1)
        span = np.zeros(NCH * NB, dtype=np.int64)
        span[valid] = idxc[ke[valid]] - base[valid] + 1
        assert span.max() <= BF, f"block span {span.max()} > BF={BF}"

        lidx = np.full(NPAD, -1.0, dtype=np.float32)
        lidx[:PC] = (idxc - np.repeat(base, P)[:PC]).astype(np.float32)
        lidx_t = lidx.reshape(NCH, NB, P).transpose(2, 0, 1).reshape(
            P, NCH * NB).astype(ml_dtypes.float8_e3m4)

        parts = []
        for gstart, gc in groups:
            blk = q8[gstart * A_max:(gstart + gc) * A_max]
            parts.append(np.ascontiguousarray(
                blk.reshape(gc, NB, P, N_IN).transpose(2, 0, 1, 3)
                   .reshape(gc * A_max, N_IN)))
        xw_i = np.concatenate(parts, axis=0)

        cst = np.zeros((P, CW), dtype=ml_dtypes.float8_e3m4)
        cst[:, IOTA_OFF:IOTA_OFF + SLOTS] = iota_row[None, :]
        cst[:, LIDX_OFF:LIDX_OFF + NCH * NB] = lidx_t
        w0b = W[0].astype(ml_dtypes.bfloat16)[:, None].view(
            ml_dtypes.float8_e3m4)
        cst[:, W0_OFF:W0_OFF + 2] = w0b
        in_maps.append({"xw": xw_i, "cst": np.ascontiguousarray(cst)})

        # unpack tables: (partition, column, molecule) per live slot
        nblk = int(valid.sum())
        sp = span[:nblk]
        tot = int(sp.sum())
        starts = np.zeros(nblk, dtype=np.int64)
        starts[1:] = np.cumsum(sp)[:-1]
        blk_of = np.repeat(np.arange(nblk, dtype=np.int64), sp)
        s_off = np.arange(tot, dtype=np.int64) - np.repeat(starts, sp)
        mol = np.repeat(base[:nblk], sp) + s_off
        slotfull = (blk_of % NB) * BF + s_off
        chunkc = blk_of // NB
        p_idx = slotfull % P
        col_idx = chunkc * NYC + slotfull // P
        unpack.append((p_idx, col_idx, mol))
    return in_maps, unpack


def _run(inputs, trace=False):
    from concourse import bass_utils

    in_maps, unpack = _prep(inputs)
    key = (NCH, BF)
    if key not in _graph_cache:
        _graph_cache[key] = _build_graph()
    nc = _graph_cache[key]

    res = bass_utils.run_bass_kernel_spmd(
        nc, in_maps, core_ids=list(range(NCORES)), trace=trace
    )
    y = np.zeros(N_MOL, dtype=np.float32)
    for i in range(NCORES):
        arr = res.results[i]["out"].reshape(P, NCH * NYC)
        p_idx, col_idx, mol = unpack[i]
        np.add.at(y, mol, arr[p_idx, col_idx])
    return y, res


def kernel(**inputs) -> np.ndarray:
    y, _ = _run(inputs, trace=False)
    return y
